# revision 28
# baseline (speedup 1.0000x reference)
"""CharRNN (2-layer BN-LSTM) Trainium2 kernel, 8-way tensor-parallel.

Strategy
--------
Shard the 4H gate dimension 8 ways: core k owns columns [k*256,(k+1)*256)
of EACH gate block (i,j,f,o) of Wx/Wh, and the matching H-columns of
c/h/gc/bc.  Batch stays full (B=256) on every core, so the per-timestep
batch-norm statistics are exact (they are per-gate-column over the full
batch).  After each recurrence step the cores all-gather their h-shards
(one 8-core AllGather per step) to rebuild the full h needed for the
next step's h @ Wh.

On-chip layout is feature-major ("transposed"): activations live as
[features(partitions), batch(free)], so BN reductions are free-axis
reduce (bn_stats) and the recurrent matmul is
  gates.T[4H_s, B] = Wh_s.T @ h  ==  matmul(lhsT=Wh_s[K,4H_s], rhs=h.T[K,B])
with K=H on partitions, exactly the layout the PE wants.

Matmuls are fp32 (4 cyc/row on TRN2 PE).  bf16 was measured to produce
~20% final error through the 64-step recurrence (error amplification
~50x); fp32r (1.5e-4/matmul on HW) gives 1.23e-2 end-to-end — usable but
thin margin, so the shipped config keeps exact fp32 matmuls.

Wire-format optimizations (the graded min-wall-minus-noop metric is
dominated by per-call PJRT input transfer, not device compute):
  - xT is shipped T-sharded (4.2 MB/core instead of 33.5 MB replicated)
    and rebuilt on device with one AllGather before phase A.
  - Wh1/Wx2/Wh2 are shipped fp16 (1.13e-2 end-to-end error, CPU-verified)
    and cast to fp32 by gpsimd DMA on load.  Wx1/Wp/x stay fp32 — adding
    them to fp16 measured 2.36e-2, over the 2e-2 gate.
  - outT is bf16 on the wire, cast back to fp32 on host.
Measured: 68.3 ms (all-fp32 replicated baseline) -> 40.0 ms, relmax 1.19e-2.

Phases: A) xg1 = x@Wx1 + per-t BN  ->  xgn1 (DRAM)
        B) 64 recurrence steps layer 1 (AllGather h per step) -> h1T
        C) xg2 = h1@Wx2 + BN -> xgn2
        D) recurrence layer 2 -> h2T
        E) out.T = Wp_s.T @ h2 + bp_s (P sharded 8x64)

Host does: embedding gather, weight column-sharding, transposes,
output assembly (cheap, memory-bound only).
"""
import numpy as np

import concourse.bass as bass
import concourse.bacc as bacc
import concourse.mybir as mybir
import concourse.tile as tile
from concourse import bass_utils

F32 = mybir.dt.float32
F32R = mybir.dt.float32r
BF16 = mybir.dt.bfloat16
AF = mybir.ActivationFunctionType
ALU = mybir.AluOpType

B, V, E, H, P = 256, 32000, 512, 2048, 512
EPS = 1e-5
NC = 8            # cores
GS = 4 * H // NC  # per-core gate shard = 1024
MT = GS // 128    # m-tiles per core = 8
HS = H // NC      # per-core h/c shard = 256
QT = HS // 128    # h/c shard tiles = 2
PS = P // NC      # per-core proj shard = 64


def build_program(T=64, mm_dt=F32, ncores=NC, use_ag=True, phases='ABCDE', reps=1,
                  f32r=False, out_bf16=True, shard_x=True, w16=False):
    KT1 = E // 128    # k-tiles for Wx1 (4)
    KT = H // 128     # k-tiles for H-sized contractions (16)

    nc = bacc.Bacc("TRN2", target_bir_lowering=False, debug=False,
                   num_devices=ncores)
    dma_cast = nc.gpsimd if mm_dt != F32 else nc.sync
    # fp32r: PE reads the same fp32 bits in relaxed-precision mode — 4x the
    # matmul rate at free-dim>=256.  Applied via bitcast at the matmul sites.
    mmc = (lambda ap: ap.bitcast(F32R)) if (f32r and mm_dt == F32) else (lambda ap: ap)

    # ---- I/O ----
    TS = T // ncores  # per-core timestep slice when shard_x
    F16 = mybir.dt.float16
    XDT = F32                    # x path stays fp32 on the wire
    WDT = F16 if w16 else mm_dt  # wire dtype for the big weight matrices
    w_dma = nc.gpsimd if w16 else nc.sync  # fp16 -> fp32 needs a casting DMA
    if shard_x:
        xT = nc.dram_tensor('xT', [TS * E, B], XDT, kind="ExternalInput")
    else:
        xT = nc.dram_tensor('xT', [T, E, B], XDT, kind="ExternalInput")
    wx1 = nc.dram_tensor('wx1', [E, GS], mm_dt, kind="ExternalInput")
    wh1 = nc.dram_tensor('wh1', [H, GS], WDT, kind="ExternalInput")
    wx2 = nc.dram_tensor('wx2', [H, GS], WDT, kind="ExternalInput")
    wh2 = nc.dram_tensor('wh2', [H, GS], WDT, kind="ExternalInput")
    wp = nc.dram_tensor('wp', [H, PS], mm_dt, kind="ExternalInput")
    gb1 = nc.dram_tensor('gb1', [3, MT, 128], F32, kind="ExternalInput")
    gb2 = nc.dram_tensor('gb2', [3, MT, 128], F32, kind="ExternalInput")
    cc1 = nc.dram_tensor('cc1', [2, QT, 128], F32, kind="ExternalInput")
    cc2 = nc.dram_tensor('cc2', [2, QT, 128], F32, kind="ExternalInput")
    bps = nc.dram_tensor('bps', [PS, 1], F32, kind="ExternalInput")
    outT = nc.dram_tensor('outT', [PS, T * B], BF16 if out_bf16 else F32,
                          kind="ExternalOutput")

    # ---- internal DRAM ----
    xgn1 = nc.dram_tensor('xgn1', [GS, T * B], F32)
    xgn2 = nc.dram_tensor('xgn2', [GS, T * B], F32)

    with tile.TileContext(nc) as tc:
        with (
            tc.tile_pool(name="consts", bufs=1) as cpool,
            tc.tile_pool(name="bigw", bufs=1) as wpool,
            tc.tile_pool(name="state", bufs=1) as spool,
            tc.tile_pool(name="hbuf", bufs=2) as hpool,
            tc.tile_pool(name="xg", bufs=3) as xgpool,
            tc.tile_pool(name="work", bufs=3) as work,
            tc.tile_pool(name="gtile", bufs=2) as gpool,
            tc.tile_pool(name="ps", bufs=6, space="PSUM") as psp,
            tc.tile_pool(name="dram", bufs=2, space="DRAM") as dramp,
        ):
            rep_box = [0]

            # ---------- small constants ----------
            gbs1 = cpool.tile([128, 3, MT], F32, tag="gb1")
            nc.sync.dma_start(gbs1[:], gb1.ap().rearrange("j m p -> p j m"))
            gbs2 = cpool.tile([128, 3, MT], F32, tag="gb2")
            nc.sync.dma_start(gbs2[:], gb2.ap().rearrange("j m p -> p j m"))
            ccs1 = cpool.tile([128, 2, QT], F32, tag="cc1")
            nc.sync.dma_start(ccs1[:], cc1.ap().rearrange("j q p -> p j q"))
            ccs2 = cpool.tile([128, 2, QT], F32, tag="cc2")
            nc.sync.dma_start(ccs2[:], cc2.ap().rearrange("j q p -> p j q"))
            bpt = cpool.tile([PS, 1], F32, tag="bps")
            nc.sync.dma_start(bpt[:], bps.ap())

            # ============================================================
            # helper: BN stats chain for one m-tile group, batched affine
            # ============================================================
            def bn_batch_affine(mv_all, gscale_ap, bias_from, nmt):
                """mv_all: [128, nmt, 2] (mean,var per m-tile).
                Returns (scale[128,nmt], nbias[128,nmt]) with
                scale = gscale * rsqrt(var+eps), nbias = bias_from - mean*scale.
                gscale_ap: [128, nmt] AP; bias_from: [128, nmt] AP or None (-> 0).
                """
                veps = work.tile([128, nmt], F32, tag="veps")
                nc.vector.tensor_scalar_add(veps[:], mv_all[:, :, 1], EPS)
                std = work.tile([128, nmt], F32, tag="std")
                nc.scalar.activation(std[:], veps[:], AF.Sqrt)
                rstd = work.tile([128, nmt], F32, tag="rstd")
                nc.vector.reciprocal(rstd[:], std[:])
                scale = work.tile([128, nmt], F32, tag="scale")
                nc.vector.tensor_mul(scale[:], rstd[:], gscale_ap)
                nbias = work.tile([128, nmt], F32, tag="nbias")
                nc.vector.tensor_mul(nbias[:], mv_all[:, :, 0], scale[:])
                if bias_from is None:
                    nc.vector.tensor_scalar_mul(nbias[:], nbias[:], -1.0)
                else:
                    nc.vector.tensor_tensor(nbias[:], bias_from, nbias[:],
                                            ALU.subtract)
                return scale, nbias

            def emit_all():
                rep = rep_box[0]
                h1T = [dramp.tile([H, B], F32, tag=f"h1Tt{t}r{rep}", bufs=1,
                                   addr_space="Shared", name=f"h1Tt{t}r{rep}")
                       for t in range(T)]
                h2T = [dramp.tile([H, B], F32, tag=f"h2Tt{t}r{rep}", bufs=1,
                                   addr_space="Shared", name=f"h2Tt{t}r{rep}")
                       for t in range(T)]
                # ============================================================
                # Phase A: xg1 = x @ Wx1_s, per-t BN -> xgn1
                # ============================================================
                wxs1 = wpool.tile([128, KT1, GS], mm_dt, tag="w")
                nc.sync.dma_start(wxs1[:], wx1.ap().rearrange("(k p) g -> p k g", p=128))

                if shard_x:
                    # rebuild full xT on device: each core contributed TS
                    # timesteps; AllGather concatenates on the row axis so
                    # timestep t lands at rows [E*t : E*(t+1)].
                    xags = dramp.tile([TS * E, B], F32, tag=f"xagin_r{rep}",
                                      bufs=1, name=f"xagin_r{rep}")
                    nc.sync.dma_start(xags[:, :], xT.ap())
                    xfull = dramp.tile([T * E, B], F32, tag=f"xfull_r{rep}",
                                       bufs=1, addr_space="Shared",
                                       name=f"xfull_r{rep}")
                    nc.gpsimd.collective_compute(
                        "AllGather", ALU.bypass,
                        replica_groups=[list(range(ncores))],
                        ins=[xags.opt()], outs=[xfull.opt()],
                    )

                for t in range(T if 'A' in phases else 0):
                    xts = xgpool.tile([128, KT1, B], mm_dt, tag="xts")
                    xsrc_t = (xfull[t * E:(t + 1) * E, :] if shard_x else xT[t])
                    dma_cast.dma_start(
                        xts[:], xsrc_t.rearrange("(k p) b -> p k b", p=128))
                    mv_all = work.tile([128, MT, 2], F32, tag="mvA")
                    pss = []
                    for pair in range(MT // 2):
                        ps = psp.tile([128, 2, B], F32, tag="g2", bufs=5)
                        pss.append(ps)
                        for j in range(2):
                            m = 2 * pair + j
                            for k in range(KT1):
                                nc.tensor.matmul(ps[:, j, :],
                                                 mmc(wxs1[:, k, m * 128:(m + 1) * 128]),
                                                 mmc(xts[:, k, :]),
                                                 start=(k == 0), stop=(k == KT1 - 1))
                            st = work.tile([128, 6], F32, tag="stA")
                            nc.vector.bn_stats(st[:], ps[:, j, :])
                            nc.vector.bn_aggr(mv_all[:, m, :], st[:])
                    scale, nbias = bn_batch_affine(
                        mv_all, gbs1[:, 0, :], gbs1[:, 2, :], MT)
                    for m in range(MT):
                        g = gpool.tile([128, B], F32, tag="gA")
                        nc.vector.tensor_scalar(
                            g[:], pss[m // 2][:, m % 2, :],
                            scale[:, m:m + 1], nbias[:, m:m + 1],
                            ALU.mult, ALU.add)
                        nc.sync.dma_start(
                            xgn1[m * 128:(m + 1) * 128, t * B:(t + 1) * B], g[:])

                # ============================================================
                # recurrence (shared for both layers)
                # ============================================================
                def recurrence(whs, xgn, gbs, ccs, houtT, ff=None):
                    cT = spool.tile([128, QT, B], F32, tag="cT")
                    hT = None

                    for t in range(T):
                        xg = xgpool.tile([128, MT, B], F32, tag="xg")
                        xsrc = xgn[:, t * B:(t + 1) * B].rearrange(
                            "(m p) n -> p m n", p=128)
                        for dc in range(2):
                            nc.sync.dma_start(xg[:, 4 * dc:4 * dc + 4, :],
                                              xsrc[:, 4 * dc:4 * dc + 4, :])
                        if t > 0:
                            mv_all = work.tile([128, MT, 2], F32, tag="mvB")
                            pss = []
                            for pair in range(MT // 2):
                                ps = psp.tile([128, 2, B], F32, tag="g2", bufs=5)
                                pss.append(ps)
                                for j in range(2):
                                    m = 2 * pair + j
                                    for k in range(KT):
                                        nc.tensor.matmul(
                                            ps[:, j, :],
                                            mmc(whs[:, k, m * 128:(m + 1) * 128]),
                                            mmc(hT[:, k, :]),
                                            start=(k == 0), stop=(k == KT - 1))
                                    st = work.tile([128, 6], F32, tag="stB")
                                    nc.vector.bn_stats(st[:], ps[:, j, :])
                                    nc.vector.bn_aggr(mv_all[:, m, :], st[:])
                            if ff is not None:
                                ff(hT, t - 1)
                            scale, nbias = bn_batch_affine(mv_all, gbs[:, 1, :],
                                                           None, MT)
                            gts = gpool.tile([128, MT, B], F32, tag="gB")
                            for m in range(MT):
                                nc.vector.scalar_tensor_tensor(
                                    gts[:, m, :], pss[m // 2][:, m % 2, :],
                                    scale[:, m:m + 1],
                                    xg[:, m, :], ALU.mult, ALU.add)
                            # f-gate bias needs +1.0 (m-tiles 4,5)
                            nbf = work.tile([128, 2], F32, tag="nbf")
                            nc.vector.tensor_scalar_add(nbf[:], nbias[:, 4:6], 1.0)
                            bias_i = lambda q: nbias[:, q:q + 1]
                            bias_j = lambda q: nbias[:, 2 + q:3 + q]
                            bias_f = lambda q: nbf[:, q:q + 1]
                            bias_o = lambda q: nbias[:, 6 + q:7 + q]
                        else:
                            # h==0: bn(h@Wh)=0, so gates = xgn_t exactly
                            gts = xg
                            bias_i = lambda q: 0.0
                            bias_j = lambda q: 0.0
                            bias_f = lambda q: 1.0
                            bias_o = lambda q: 0.0
                        # gate activations: i=m0-1, j=m2-3, f=m4-5, o=m6-7
                        sigi = gpool.tile([128, QT, B], F32, tag="sigi")
                        tnj = gpool.tile([128, QT, B], F32, tag="tnj")
                        sigo = gpool.tile([128, QT, B], F32, tag="sigo")
                        if t > 0:
                            sigf = gpool.tile([128, QT, B], F32, tag="sigf")
                        for q in range(QT):
                            nc.scalar.activation(sigi[:, q, :], gts[:, q, :],
                                                 AF.Sigmoid, bias=bias_i(q))
                            nc.scalar.activation(tnj[:, q, :], gts[:, 2 + q, :],
                                                 AF.Tanh, bias=bias_j(q))
                            if t > 0:
                                nc.scalar.activation(sigf[:, q, :], gts[:, 4 + q, :],
                                                     AF.Sigmoid, bias=bias_f(q))
                            nc.scalar.activation(sigo[:, q, :], gts[:, 6 + q, :],
                                                 AF.Sigmoid, bias=bias_o(q))
                        # c' = sigf*c + sigi*tanh(j)   (t=0: c==0)
                        t2 = gpool.tile([128, QT, B], F32, tag="t2")
                        nc.vector.tensor_mul(t2[:], sigi[:], tnj[:])
                        if t > 0:
                            t1 = gpool.tile([128, QT, B], F32, tag="t1")
                            nc.vector.tensor_mul(t1[:], sigf[:], cT[:])
                            nc.vector.tensor_tensor(cT[:], t1[:], t2[:], ALU.add)
                        else:
                            nc.vector.tensor_copy(cT[:], t2[:])
                        # h' = sigo * tanh(gc*bn(c') + bc)
                        mvc = work.tile([128, QT, 2], F32, tag="mvc")
                        for q in range(QT):
                            stc = work.tile([128, 6], F32, tag="stc")
                            nc.vector.bn_stats(stc[:], cT[:, q, :])
                            nc.vector.bn_aggr(mvc[:, q, :], stc[:])
                        scale_c, bias_c = bn_batch_affine(
                            mvc, ccs[:, 0, :], ccs[:, 1, :], QT)
                        hsh = gpool.tile([128, QT, B], F32, tag="hsh")
                        for q in range(QT):
                            nc.scalar.activation(hsh[:, q, :], cT[:, q, :], AF.Tanh,
                                                 bias=bias_c[:, q:q + 1],
                                                 scale=scale_c[:, q:q + 1])
                        nc.vector.tensor_mul(hsh[:], sigo[:], hsh[:])
                        # publish shard, all-gather full h_t
                        agin = dramp.tile([HS, B], F32, tag="agin")
                        nc.sync.dma_start(
                            agin.rearrange("(q p) b -> p q b", p=128), hsh[:])
                        if use_ag:
                            nc.gpsimd.collective_compute(
                                "AllGather", ALU.bypass,
                                replica_groups=[list(range(ncores))],
                                ins=[agin.opt()], outs=[houtT[t].opt()],
                            )
                        else:
                            # timing-only stand-in for the AllGather (numerics
                            # wrong): copy the local shard into its slot
                            nc.sync.dma_start(houtT[t][0:HS, :], agin[:, :])
                        hT = hpool.tile([128, KT, B], mm_dt, tag="hT")
                        hsrc = houtT[t].rearrange("(k p) b -> p k b", p=128)
                        for dc in range(4):
                            dma_cast.dma_start(hT[:, 4 * dc:4 * dc + 4, :],
                                               hsrc[:, 4 * dc:4 * dc + 4, :])
                    if ff is not None:
                        ff(hT, T - 1)

                # ============================================================
                # Phase B: layer-1 recurrence
                # ============================================================
                if 'B' in phases:
                    whs1 = wpool.tile([128, KT, GS], mm_dt, tag="w")
                    wsrc = wh1.ap().rearrange("(k p) g -> p k g", p=128)
                    for dc in range(4):
                        w_dma.dma_start(whs1[:, 4 * dc:4 * dc + 4, :],
                                        wsrc[:, 4 * dc:4 * dc + 4, :])
                    recurrence(whs1, xgn1, gbs1, ccs1, h1T)

                # ============================================================
                # Phase C: xg2 = h1 @ Wx2_s, BN -> xgn2
                # ============================================================
                wxs2 = wpool.tile([128, KT, GS], mm_dt, tag="w")
                wsrc = wx2.ap().rearrange("(k p) g -> p k g", p=128)
                for dc in range(4):
                    w_dma.dma_start(wxs2[:, 4 * dc:4 * dc + 4, :],
                                    wsrc[:, 4 * dc:4 * dc + 4, :])
                for t in range(T if 'C' in phases else 0):
                    hts = hpool.tile([128, KT, B], mm_dt, tag="hT")
                    hsrc = h1T[t].rearrange("(k p) b -> p k b", p=128)
                    for dc in range(4):
                        dma_cast.dma_start(hts[:, 4 * dc:4 * dc + 4, :],
                                           hsrc[:, 4 * dc:4 * dc + 4, :])
                    mv_all = work.tile([128, MT, 2], F32, tag="mvA")
                    pss = []
                    for pair in range(MT // 2):
                        ps = psp.tile([128, 2, B], F32, tag="g2", bufs=5)
                        pss.append(ps)
                        for j in range(2):
                            m = 2 * pair + j
                            for k in range(KT):
                                nc.tensor.matmul(ps[:, j, :],
                                                 mmc(wxs2[:, k, m * 128:(m + 1) * 128]),
                                                 mmc(hts[:, k, :]),
                                                 start=(k == 0), stop=(k == KT - 1))
                            st = work.tile([128, 6], F32, tag="stA")
                            nc.vector.bn_stats(st[:], ps[:, j, :])
                            nc.vector.bn_aggr(mv_all[:, m, :], st[:])
                    scale, nbias = bn_batch_affine(
                        mv_all, gbs2[:, 0, :], gbs2[:, 2, :], MT)
                    for m in range(MT):
                        g = gpool.tile([128, B], F32, tag="gA")
                        nc.vector.tensor_scalar(
                            g[:], pss[m // 2][:, m % 2, :],
                            scale[:, m:m + 1], nbias[:, m:m + 1],
                            ALU.mult, ALU.add)
                        nc.sync.dma_start(
                            xgn2[m * 128:(m + 1) * 128, t * B:(t + 1) * B], g[:])

                # ============================================================
                # Phase D: layer-2 recurrence, projection folded in (phase E)
                # ============================================================
                if 'D' in phases:
                    wps = wpool.tile([128, KT, PS], mm_dt, tag="wp")
                    nc.sync.dma_start(
                        wps[:], wp.ap().rearrange("(k p) s -> p k s", p=128))

                    def proj_ff(hT_tile, t):
                        ps = psp.tile([PS, B], F32, tag="psE", bufs=2)
                        for k in range(KT):
                            nc.tensor.matmul(ps[:], mmc(wps[:, k, :]),
                                             mmc(hT_tile[:, k, :]),
                                             start=(k == 0), stop=(k == KT - 1))
                        o = gpool.tile([PS, B], BF16 if out_bf16 else F32,
                                       tag="oE")
                        nc.vector.tensor_scalar_add(o[:], ps[:], bpt[:, 0:1])
                        nc.sync.dma_start(outT[:, t * B:(t + 1) * B], o[:])

                    whs2 = wpool.tile([128, KT, GS], mm_dt, tag="w")
                    wsrc = wh2.ap().rearrange("(k p) g -> p k g", p=128)
                    for dc in range(4):
                        w_dma.dma_start(whs2[:, 4 * dc:4 * dc + 4, :],
                                        wsrc[:, 4 * dc:4 * dc + 4, :])
                    recurrence(whs2, xgn2, gbs2, ccs2, h2T,
                               ff=proj_ff if 'E' in phases else None)


            for _rep in range(reps):
                rep_box[0] = _rep
                emit_all()


    nc.compile()
    return nc


def shard_inputs(inputs, T=64):
    """Host-side prep: embedding gather, transposes, per-core weight shards."""
    ii = {k: np.asarray(v) for k, v in inputs.items()}
    x = ii['embedding'][ii['input_data']]          # [B, T, E] f32
    xT = np.ascontiguousarray(x.transpose(1, 2, 0))  # [T, E, B]
    TS = T // NC  # per-core timestep slice (xT shipped sharded, AG'd on device)

    def gate_shard(w, k):       # w: [K, 4H] -> [K, GS]
        K = w.shape[0]
        return np.ascontiguousarray(
            w.reshape(K, 4, NC, HS)[:, :, k, :].reshape(K, GS))

    def vec_shard(v, k):        # v: [4H] -> [GS]
        return np.ascontiguousarray(v.reshape(4, NC, HS)[:, k, :].reshape(GS))

    in_maps = []
    for k in range(NC):
        gb1 = np.stack([vec_shard(ii['gx1'], k), vec_shard(ii['gh1'], k),
                        vec_shard(ii['b1'], k)]).reshape(3, MT, 128)
        gb2 = np.stack([vec_shard(ii['gx2'], k), vec_shard(ii['gh2'], k),
                        vec_shard(ii['b2'], k)]).reshape(3, MT, 128)
        cc1 = np.stack([ii['gc1'][k * HS:(k + 1) * HS],
                        ii['bc1'][k * HS:(k + 1) * HS]]).reshape(2, QT, 128)
        cc2 = np.stack([ii['gc2'][k * HS:(k + 1) * HS],
                        ii['bc2'][k * HS:(k + 1) * HS]]).reshape(2, QT, 128)
        in_maps.append({
            'xT': np.ascontiguousarray(
                xT[k * TS:(k + 1) * TS].reshape(TS * E, B)),
            'wx1': gate_shard(ii['Wx1'], k),
            'wh1': gate_shard(ii['Wh1'], k).astype(np.float16),
            'wx2': gate_shard(ii['Wx2'], k).astype(np.float16),
            'wh2': gate_shard(ii['Wh2'], k).astype(np.float16),
            'wp': np.ascontiguousarray(ii['Wp'][:, k * PS:(k + 1) * PS]),
            'gb1': np.ascontiguousarray(gb1),
            'gb2': np.ascontiguousarray(gb2),
            'cc1': np.ascontiguousarray(cc1),
            'cc2': np.ascontiguousarray(cc2),
            'bps': np.ascontiguousarray(
                ii['bp'][k * PS:(k + 1) * PS].reshape(PS, 1)),
        })
    return in_maps


def assemble_output(results, T=64):
    """results: list of 8 per-core dicts with 'outT' [PS, T*B]."""
    full = np.concatenate([np.asarray(r['outT'], dtype=np.float32)
                           for r in results], axis=0)            # [P, T*B]
    full = full.reshape(P, T, B).transpose(2, 1, 0)              # [B, T, P]
    return np.ascontiguousarray(full.reshape(B * T, P))


def kernel(**inputs):
    T = int(np.asarray(inputs['input_data']).shape[1])
    nc = build_program(T=T, w16=True)
    in_maps = shard_inputs(inputs, T=T)
    res = bass_utils.run_bass_kernel_spmd(
        nc, in_maps, core_ids=list(range(NC)))
    return assemble_output(res.results, T=T)

